# revision 32
# baseline (speedup 1.0000x reference)
"""Trainium2 Bass kernel for AccentVarianceAdaptor (v6: on-chip one-hot gather).

Computation (per batch row):
  pbin = searchsorted(linspace(50,400,256), clip(pitch,50,400), 'left')
  ebin = searchsorted(linspace(0,1,256),  clip(energy,0,1),  'left')
  y    = encoder + ptab[pbin] + etab[ebin]               # [S, H]
  dur  = max(round(duration), 1); cum = cumsum(dur)
  out[t] = y[searchsorted(cum, t, 'right')] * (t < cum[-1])  # [T, H]

Mapping to the hardware (one NeuronCore handles 4 batch rows):
  - table lookup: C[bin, tok] = (boundary[bin] < v[tok]) built by DMA-
    replicating the value rows across partitions + tensor_scalar(is_gt);
    y = sum C_half.T @ dTab (telescoping sum == exact row select, bf16
    dTab - tolerance allows); base table rows are folded into enc on host.
  - durations: (d + 2^23) - 2^23 rounds half-to-even exactly in f32;
    cumulative sums via per-partition tensor_tensor_scan in a transposed
    [rows, 128] layout + a tiny matmul for cross-row offsets (all operand
    values stay fp16-exact; sums accumulate in f32 PSUM).
  - length-regulate entirely on-chip (no HBM scratch, no dma_gather):
    scatter 1.0 at delta[cum[j]]; frame_idx = scan of delta; idx row is
    assembled to [1, T] and DMA-replicated to bc_all [128, T];
    C_onehot[j, t] = is_equal(idx_t, 128k + j) in fp16 (one op per token
    chunk); out[frame_tile] = sum_k C_onehot_k.T @ y_k as fp16 matmuls.
    Since dur in [1,8], idx_t in [floor(t/8), min(t,511)], so most
    (token-chunk, frame-tile) blocks are statically zero and skipped
    (74 of 128 matmuls per batch row survive).
"""

import os
import sys

for _p in ("/opt/trn_rl_repo", "/root/.axon_site/_ro/trn_rl_repo"):
    if os.path.isdir(_p) and _p not in sys.path:
        sys.path.insert(0, _p)

import numpy as np

from concourse import bacc, mybir, tile
from concourse.bass import AP, IndirectOffsetOnAxis, ts
from concourse.bass_utils import run_bass_kernel_spmd

B, S, H = 32, 512, 256
NBINS = 256
T = 4096
NCORES = 8
BPC = B // NCORES  # batches per core
P = 128
NCH = S // P  # token chunks per batch (4)
DELTA_N = T + 8
F32 = mybir.dt.float32
F16 = mybir.dt.float16
BF16 = mybir.dt.bfloat16
I32 = mybir.dt.int32
A = mybir.AluOpType
AF = mybir.ActivationFunctionType

# (name, rows, cols) layout of the packed f32 constant block
_PACKF = [("bndp", P, 2), ("bnde", P, 2), ("ones1", 1, P), ("onecol", P, 1),
          ("tokid", P, NCH)]
PACKF_COLS = sum(c for _, _, c in _PACKF)
# fp16 constant block
_PACKH = [("slt4h", NCH, NCH), ("slt32h", 32, 32), ("identh", P, P)]
PACKH_COLS = sum(c for _, _, c in _PACKH)


def _boundaries():
    """Bit-exact copies of the f32 boundaries the jax reference uses."""
    import jax

    with jax.default_device(jax.devices("cpu")[0]):
        import jax.numpy as jnp

        bp = np.asarray(jnp.linspace(50.0, 400.0, NBINS), np.float32)
        be = np.asarray(jnp.linspace(0.0, 1.0, NBINS), np.float32)
    return bp, be


def _host_constants(pitch_table, energy_table):
    bp, be = _boundaries()
    consts = {}
    import ml_dtypes
    for name, tab in (("dpt", pitch_table), ("det", energy_table)):
        d = np.zeros((NBINS, H), np.float32)
        d[:-1] = tab[1:] - tab[:-1]  # f32 arithmetic, row 255 stays 0
        consts[name + "_hi"] = d.astype(ml_dtypes.bfloat16)
    j = np.arange(P, dtype=np.float32)
    c4 = np.arange(NCH, dtype=np.float32)
    c32 = np.arange(32, dtype=np.float32)
    valsf = {
        "bndp": bp.reshape(2, P).T.copy(),  # [128, 2], col h = b[h*128 + p]
        "bnde": be.reshape(2, P).T.copy(),
        "ones1": np.ones((1, P), np.float32),
        "onecol": np.ones((P, 1), np.float32),
        "tokid": j[:, None] + 128.0 * c4[None, :],
    }
    pk = np.zeros((P, PACKF_COLS), np.float32)
    c0 = 0
    for name, rows, cols in _PACKF:
        pk[:rows, c0 : c0 + cols] = valsf[name]
        c0 += cols
    consts["pconstf"] = pk
    valsh = {
        "slt4h": (c4[:, None] < c4[None, :]),
        "slt32h": (c32[:, None] < c32[None, :]),
        "identh": np.eye(P),
    }
    pkh = np.zeros((P, PACKH_COLS), np.float16)
    c0 = 0
    for name, rows, cols in _PACKH:
        pkh[:rows, c0 : c0 + cols] = valsh[name].astype(np.float16)
        c0 += cols
    consts["pconsth"] = pkh
    return consts


def _chunk_range(ft):
    """Token chunks that can feed frame tile ft (dur in [1,8])."""
    return range(ft // 8, min(NCH - 1, ft) + 1)


def build_nc():
    nc = bacc.Bacc("TRN2", target_bir_lowering=False, debug=False, enable_asserts=False)

    enc_dr = nc.dram_tensor("enc", [BPC, S, H], F16, kind="ExternalInput")
    pit_dr = nc.dram_tensor("pitch", [BPC, S], F32, kind="ExternalInput")
    ene_dr = nc.dram_tensor("energy", [BPC, S], F32, kind="ExternalInput")
    dur_dr = nc.dram_tensor("durt", [BPC, S], F32, kind="ExternalInput")
    tab_dr = {
        nm: nc.dram_tensor(nm, [NBINS, H], BF16, kind="ExternalInput")
        for nm in ("dpt_hi", "det_hi")
    }
    pkf_dr = nc.dram_tensor("pconstf", [P, PACKF_COLS], F32, kind="ExternalInput")
    pkh_dr = nc.dram_tensor("pconsth", [P, PACKH_COLS], F16, kind="ExternalInput")
    out_dr = [
        nc.dram_tensor(f"out{b}", [T, H], F32, kind="ExternalOutput")
        for b in range(BPC)
    ]
    delta_dr = [[nc.dram_tensor(f"delta{b}_{h}", [DELTA_N, 1], F32) for h in range(2)]
                for b in range(BPC)]
    idxr_dr = [nc.dram_tensor(f"idxr{b}", [1, T], F16) for b in range(BPC)]

    with tile.TileContext(nc) as tc:
        with (
            tc.tile_pool(name="const", bufs=1) as cp,
            tc.tile_pool(name="work", bufs=2) as wp,
            tc.tile_pool(name="inb", bufs=1) as ib,
            tc.tile_pool(name="ytil", bufs=3) as yp,
            tc.tile_pool(name="cmat", bufs=3) as cpl,
            tc.tile_pool(name="gat", bufs=3) as gp,
            tc.tile_pool(name="pout", bufs=2, space="PSUM") as po,
            tc.tile_pool(name="peps", bufs=2, space="PSUM") as pe,
            tc.tile_pool(name="psmall", bufs=2, space="PSUM") as psm,
        ):
            # ---- constants (one packed DMA per dtype) ----
            csb = {}
            pkf_sb = cp.tile([P, PACKF_COLS], F32, tag="pconstf")
            nc.sync.dma_start(out=pkf_sb[:], in_=pkf_dr[:])
            c0 = 0
            for name, rows, cols in _PACKF:
                csb[name] = pkf_sb[0:rows, c0 : c0 + cols]
                c0 += cols
            pkh_sb = cp.tile([P, PACKH_COLS], F16, tag="pconsth")
            nc.sync.dma_start(out=pkh_sb[:], in_=pkh_dr[:])
            c0 = 0
            for name, rows, cols in _PACKH:
                csb[name] = pkh_sb[0:rows, c0 : c0 + cols]
                c0 += cols
            for nm, dr in tab_dr.items():
                t_ = cp.tile([P, 2, H], BF16, tag=nm)
                nc.sync.dma_start(
                    out=t_[:], in_=dr[:].rearrange("(h p) f -> p h f", p=P)
                )
                csb[nm] = t_
            zt = cp.tile([24, 171], F32)  # 24*171 == DELTA_N
            nc.gpsimd.memset(zt[:], 0.0)
            for b in range(BPC):
                for h in range(2):
                    nc.gpsimd.dma_start(
                        out=delta_dr[b][h][:].rearrange("(r q) o -> r (q o)", r=24),
                        in_=zt[:],
                    )

            # ---- early input prefetch; zeros + dur first on sync queue ----
            enc_sb, vp_reps, ve_reps, dur_raws = {}, {}, {}, {}
            for b in range(BPC):
                dr_ = ib.tile([NCH, P], F32, tag=f"dw{b}")
                nc.sync.dma_start(
                    out=dr_[:], in_=dur_dr[b].rearrange("(c p) -> c p", p=P)
                )
                dur_raws[b] = dr_

            e0 = ib.tile([P, NCH, H], F16, tag="enc0")
            nc.sync.dma_start(
                out=e0[:], in_=enc_dr[0].rearrange("(c p) f -> p c f", p=P)
            )
            enc_sb[0] = e0
            for b in range(BPC):
                vp_ = ib.tile([P, S], F32, tag=f"vp{b}")
                ve_ = ib.tile([P, S], F32, tag=f"ve{b}")
                nc.sync.dma_start(
                    out=vp_[:], in_=pit_dr[b][None, :].to_broadcast([P, S])
                )
                nc.sync.dma_start(
                    out=ve_[:], in_=ene_dr[b][None, :].to_broadcast([P, S])
                )
                vp_reps[b], ve_reps[b] = vp_, ve_
            for b in range(1, BPC):
                e_ = ib.tile([P, NCH, H], F16, tag=f"enc{b}")
                nc.sync.dma_start(
                    out=e_[:], in_=enc_dr[b].rearrange("(c p) f -> p c f", p=P)
                )
                enc_sb[b] = e_

            c_tiles = {}
            y_tiles = {}

            def phase0(b):
                # ---- dur = max(round_half_even(durt), 1), [4, 128] fp16 ----
                MAGIC = float(1 << 23)
                dr0 = wp.tile([NCH, P], F32, tag="dr0")
                nc.vector.tensor_scalar(out=dr0[:], in0=dur_raws[b][:], scalar1=MAGIC, scalar2=MAGIC, op0=A.add, op1=A.subtract)
                dur4 = wp.tile([NCH, P], F16, tag="dur")
                nc.vector.tensor_scalar(out=dur4[:], in0=dr0[:], scalar1=1.0, scalar2=None, op0=A.max)

                # ---- per-row inclusive scan; cross-row offsets via matmul ----
                loc = wp.tile([NCH, P], F16, tag="loc")
                nc.vector.tensor_tensor_scan(out=loc[:], data0=dur4[:], data1=dur4[:], initial=0.0, op0=A.add, op1=A.bypass)
                # off_row[0, c] = sum_{k<c} rowtot[k]  (rowtot = loc[:, 127])
                offr_ps = psm.tile([1, NCH], F32, tag="small")
                nc.tensor.matmul(out=offr_ps[:], lhsT=loc[:, P - 1 : P], rhs=csb["slt4h"], start=True, stop=True)
                off_sb = wp.tile([1, NCH], F32, tag="off4")
                nc.vector.tensor_copy(out=off_sb[:], in_=offr_ps[:])
                # cum[p, c] = loc[c, p] + off[c]
                cum_ps = psm.tile([P, NCH], F32, tag="small")
                nc.tensor.matmul(out=cum_ps[:], lhsT=loc[:], rhs=csb["identh"][0:NCH, 0:NCH], start=True, stop=False)
                nc.tensor.matmul(out=cum_ps[:], lhsT=csb["ones1"], rhs=off_sb[:], start=False, stop=True)
                cum_i32 = wp.tile([P, NCH], I32, tag="cumi")
                nc.vector.tensor_copy(out=cum_i32[:], in_=cum_ps[:])

                # ---- delta: scatter 1.0 at cum positions (zeroed up-front) ----
                for c in range(NCH):
                    nc.gpsimd.indirect_dma_start(
                        out=delta_dr[b][c // 2][:],
                        out_offset=IndirectOffsetOnAxis(ap=cum_i32[:, c : c + 1], axis=0),
                        in_=csb["onecol"],
                        in_offset=None,
                    )

            def phase_idx(b):
                # ---- frame_idx = inclusive prefix of delta, [32, 128] fp16;
                # the scan sums both half-delta streams as it scans ----
                d32a = wp.tile([32, P], F32, tag="d32a")
                nc.scalar.dma_start(
                    out=d32a[:],
                    in_=delta_dr[b][0][0:T, :].rearrange("(r p) o -> r (p o)", p=P),
                )
                d32b = wp.tile([32, P], F32, tag="d32b")
                nc.scalar.dma_start(
                    out=d32b[:],
                    in_=delta_dr[b][1][0:T, :].rearrange("(r p) o -> r (p o)", p=P),
                )
                loc = wp.tile([32, P], F16, tag="floc")
                nc.vector.tensor_tensor_scan(out=loc[:], data0=d32a[:], data1=d32b[:], initial=0.0, op0=A.add, op1=A.add)
                # off_col[r, 0] = sum_{k<r} rowtot[k]
                offc_ps = psm.tile([32, 1], F32, tag="small")
                nc.tensor.matmul(out=offc_ps[:], lhsT=csb["slt32h"], rhs=loc[:, P - 1 : P], start=True, stop=True)
                offc_sb = wp.tile([32, 1], F32, tag="off32")
                nc.vector.tensor_copy(out=offc_sb[:], in_=offc_ps[:])
                idxT = wp.tile([32, P], F16, tag="idxT")
                nc.vector.tensor_scalar(out=idxT[:], in0=loc[:], scalar1=offc_sb[:], scalar2=None, op0=A.add)
                # assemble [1, 4096] row in DRAM, then replicate to 128 partitions
                nc.scalar.dma_start(
                    out=idxr_dr[b][0:1, :].rearrange("o (r p) -> o r p", p=P),
                    in_=idxT[:],
                )
                bc_all = wp.tile([P, T], F16, tag="bca")
                nc.scalar.dma_start(
                    out=bc_all[:], in_=idxr_dr[b][0][None, :].to_broadcast([P, T])
                )
                # ---- one-hot C per token chunk ----
                for k in range(NCH):
                    ftlo, fthi = k, min(8 * k + 7, 31)
                    span = (fthi - ftlo + 1) * P
                    ct = cpl.tile([P, span], F16, tag=f"ck{k}")
                    nc.vector.tensor_scalar(
                        out=ct[:], in0=bc_all[:, ftlo * P : (fthi + 1) * P],
                        scalar1=csb["tokid"][:, k : k + 1], scalar2=None,
                        op0=A.is_equal,
                    )
                    c_tiles[(b, k)] = ct

            cmp_tiles = {}

            def phase_cmp(b):
                # ---- C matrices: C[bin_p, tok] = (boundary[bin] < v[tok]) ----
                cp0 = ib.tile([P, S], BF16, tag=f"cp0_{b}")
                cp1 = ib.tile([P, S], BF16, tag=f"cp1_{b}")
                nc.vector.tensor_scalar(out=cp0[:], in0=vp_reps[b][:], scalar1=csb["bndp"][:, 0:1], scalar2=None, op0=A.is_gt)
                nc.vector.tensor_scalar(out=cp1[:], in0=vp_reps[b][:], scalar1=csb["bndp"][:, 1:2], scalar2=None, op0=A.is_gt)
                ce0 = ib.tile([P, S], BF16, tag=f"ce0_{b}")
                ce1 = ib.tile([P, S], BF16, tag=f"ce1_{b}")
                nc.vector.tensor_scalar(out=ce0[:], in0=ve_reps[b][:], scalar1=csb["bnde"][:, 0:1], scalar2=None, op0=A.is_gt)
                nc.vector.tensor_scalar(out=ce1[:], in0=ve_reps[b][:], scalar1=csb["bnde"][:, 1:2], scalar2=None, op0=A.is_gt)
                cmp_tiles[b] = (cp0, cp1, ce0, ce1)

            def phase_y(b):
                # ---- y = (enc + base) + dtab telescoping, fp16 ----
                cp0, cp1, ce0, ce1 = cmp_tiles[b]
                y_sb = yp.tile([P, NCH, H], F16, tag="y")
                for c in range(NCH):
                    eps = pe.tile([P, H], F32, tag="eps")
                    nc.tensor.matmul(out=eps[:], lhsT=cp0[:, ts(c, P)], rhs=csb["dpt_hi"][:, 0, :], start=True, stop=False)
                    nc.tensor.matmul(out=eps[:], lhsT=cp1[:, ts(c, P)], rhs=csb["dpt_hi"][:, 1, :], start=False, stop=False)
                    nc.tensor.matmul(out=eps[:], lhsT=ce0[:, ts(c, P)], rhs=csb["det_hi"][:, 0, :], start=False, stop=False)
                    nc.tensor.matmul(out=eps[:], lhsT=ce1[:, ts(c, P)], rhs=csb["det_hi"][:, 1, :], start=False, stop=False)
                    nc.tensor.matmul(out=eps[:], lhsT=csb["identh"], rhs=enc_sb[b][:, c, :], start=False, stop=True)
                    if c % 2 == 0:
                        nc.vector.tensor_copy(out=y_sb[:, c, :], in_=eps[:])
                    else:
                        nc.scalar.activation(out=y_sb[:, c, :], in_=eps[:], func=AF.Copy)
                y_tiles[b] = y_sb

            def phase_out(b, g4s=(0, 1, 2, 3)):
                # ---- out[ft*128+p, :] = y[idx, :] via one-hot fp16 matmuls ----
                y_sb = y_tiles[b]
                for g4 in g4s:  # 1024-frame store groups
                    gbuf = gp.tile([P, 8, H], F32, tag="g")
                    for q in range(2):  # quads of frame tiles
                        out_ps = po.tile([P, 1024], F32, tag="out")
                        for half in range(4):
                            ft = g4 * 8 + q * 4 + half
                            ks = list(_chunk_range(ft))
                            for j, k in enumerate(ks):
                                nc.tensor.matmul(
                                    out=out_ps[:, half * H : (half + 1) * H],
                                    lhsT=c_tiles[(b, k)][:, (ft - k) * P : (ft - k + 1) * P],
                                    rhs=y_sb[:, k, :],
                                    start=(j == 0), stop=(j == len(ks) - 1),
                                )
                        if (g4 * 2 + q) % 3 == 0:
                            nc.vector.tensor_copy(out=gbuf[:, 4 * q : 4 * q + 4, :], in_=out_ps[:])
                        else:
                            nc.scalar.activation(
                                out=gbuf[:, 4 * q : 4 * q + 4, :], in_=out_ps[:], func=AF.Copy
                            )
                    if b == BPC - 1:
                        nc.sync.dma_start(
                            out=out_dr[b][g4 * 1024 : g4 * 1024 + 512, :].rearrange(
                                "(c p) f -> p c f", p=P
                            ),
                            in_=gbuf[:, 0:4, :],
                        )
                        nc.scalar.dma_start(
                            out=out_dr[b][g4 * 1024 + 512 : (g4 + 1) * 1024, :].rearrange(
                                "(c p) f -> p c f", p=P
                            ),
                            in_=gbuf[:, 4:8, :],
                        )
                    else:
                        nc.sync.dma_start(
                            out=out_dr[b][g4 * 1024 : (g4 + 1) * 1024, :].rearrange(
                                "(c p) f -> p c f", p=P
                            ),
                            in_=gbuf[:],
                        )

            for b in range(BPC):
                phase0(b)
            phase_cmp(0)
            phase_y(0)
            phase_idx(0)
            phase_cmp(1)
            phase_y(1)
            phase_idx(1)
            phase_out(0)
            phase_cmp(2)
            phase_y(2)
            phase_idx(2)
            phase_out(1)
            phase_cmp(3)
            phase_y(3)
            phase_idx(3)
            phase_out(2)
            phase_out(3)

    nc.compile()
    return nc


_NC_CACHE = {}


def _get_nc():
    if "nc" not in _NC_CACHE:
        _NC_CACHE["nc"] = build_nc()
    return _NC_CACHE["nc"]


def make_in_maps(inputs):
    enc = np.asarray(inputs["encoder_output"], np.float32)
    pit = np.ascontiguousarray(np.asarray(inputs["pitch_target"], np.float32))
    ene = np.ascontiguousarray(np.asarray(inputs["energy_target"], np.float32))
    dur = np.ascontiguousarray(np.asarray(inputs["duration_target"], np.float32))
    ptab = np.asarray(inputs["pitch_table"], np.float32)
    etab = np.asarray(inputs["energy_table"], np.float32)
    # fold the telescoping-sum base row into enc on the host
    enc = np.ascontiguousarray((enc + (ptab[0] + etab[0])[None, None, :]).astype(np.float16))
    consts = _host_constants(ptab, etab)
    in_maps = []
    for c in range(NCORES):
        sl = slice(c * BPC, (c + 1) * BPC)
        m = dict(consts)
        m["enc"] = enc[sl]
        m["pitch"] = pit[sl]
        m["energy"] = ene[sl]
        m["durt"] = dur[sl]
        in_maps.append(m)
    return in_maps


def run(inputs, trace=False):
    nc = _get_nc()
    in_maps = make_in_maps(inputs)
    res = run_bass_kernel_spmd(nc, in_maps, list(range(NCORES)), trace=trace)
    out = np.empty((B, T, H), np.float32)
    for c in range(NCORES):
        for b in range(BPC):
            out[c * BPC + b] = res.results[c][f"out{b}"]
    return out, res


def kernel(**inputs):
    out, _ = run(inputs, trace=False)
    return out


# revision 33
# speedup vs baseline: 1.0621x; 1.0621x over previous
"""Trainium2 Bass kernel for AccentVarianceAdaptor (v6: on-chip one-hot gather).

Computation (per batch row):
  pbin = searchsorted(linspace(50,400,256), clip(pitch,50,400), 'left')
  ebin = searchsorted(linspace(0,1,256),  clip(energy,0,1),  'left')
  y    = encoder + ptab[pbin] + etab[ebin]               # [S, H]
  dur  = max(round(duration), 1); cum = cumsum(dur)
  out[t] = y[searchsorted(cum, t, 'right')] * (t < cum[-1])  # [T, H]

Mapping to the hardware (one NeuronCore handles 4 batch rows):
  - table lookup: C[bin, tok] = (boundary[bin] < v[tok]) built by DMA-
    replicating the value rows across partitions + tensor_scalar(is_gt);
    y = sum C_half.T @ dTab (telescoping sum == exact row select, bf16
    dTab - tolerance allows); base table rows are folded into enc on host.
  - durations: (d + 2^23) - 2^23 rounds half-to-even exactly in f32;
    cumulative sums via per-partition tensor_tensor_scan in a transposed
    [rows, 128] layout + a tiny matmul for cross-row offsets (all operand
    values stay fp16-exact; sums accumulate in f32 PSUM).
  - length-regulate entirely on-chip (no HBM scratch, no dma_gather):
    scatter 1.0 at delta[cum[j]]; frame_idx = scan of delta; idx row is
    assembled to [1, T] and DMA-replicated to bc_all [128, T];
    C_onehot[j, t] = is_equal(idx_t, 128k + j) in fp16 (one op per token
    chunk); out[frame_tile] = sum_k C_onehot_k.T @ y_k as fp16 matmuls.
    Since dur in [1,8], idx_t in [floor(t/8), min(t,511)], so most
    (token-chunk, frame-tile) blocks are statically zero and skipped
    (74 of 128 matmuls per batch row survive).
"""

import os
import sys

for _p in ("/opt/trn_rl_repo", "/root/.axon_site/_ro/trn_rl_repo"):
    if os.path.isdir(_p) and _p not in sys.path:
        sys.path.insert(0, _p)

import numpy as np

from concourse import bacc, mybir, tile
from concourse.bass import AP, IndirectOffsetOnAxis, ts
from concourse.bass_utils import run_bass_kernel_spmd

B, S, H = 32, 512, 256
NBINS = 256
T = 4096
NCORES = 8
BPC = B // NCORES  # batches per core
P = 128
NCH = S // P  # token chunks per batch (4)
DELTA_N = T + 8
F32 = mybir.dt.float32
F16 = mybir.dt.float16
BF16 = mybir.dt.bfloat16
I32 = mybir.dt.int32
A = mybir.AluOpType
AF = mybir.ActivationFunctionType

# (name, rows, cols) layout of the packed f32 constant block
_PACKF = [("bndp", P, 2), ("bnde", P, 2), ("ones1", 1, P), ("onecol", P, 1),
          ("tokid", P, NCH)]
PACKF_COLS = sum(c for _, _, c in _PACKF)
# fp16 constant block
_PACKH = [("slt4h", NCH, NCH), ("slt32h", 32, 32), ("identh", P, P)]
PACKH_COLS = sum(c for _, _, c in _PACKH)


def _boundaries():
    """Bit-exact copies of the f32 boundaries the jax reference uses."""
    import jax

    with jax.default_device(jax.devices("cpu")[0]):
        import jax.numpy as jnp

        bp = np.asarray(jnp.linspace(50.0, 400.0, NBINS), np.float32)
        be = np.asarray(jnp.linspace(0.0, 1.0, NBINS), np.float32)
    return bp, be


def _host_constants(pitch_table, energy_table):
    bp, be = _boundaries()
    consts = {}
    import ml_dtypes
    for name, tab in (("dpt", pitch_table), ("det", energy_table)):
        d = np.zeros((NBINS, H), np.float32)
        d[:-1] = tab[1:] - tab[:-1]  # f32 arithmetic, row 255 stays 0
        consts[name + "_hi"] = d.astype(ml_dtypes.bfloat16)
    j = np.arange(P, dtype=np.float32)
    c4 = np.arange(NCH, dtype=np.float32)
    c32 = np.arange(32, dtype=np.float32)
    valsf = {
        "bndp": bp.reshape(2, P).T.copy(),  # [128, 2], col h = b[h*128 + p]
        "bnde": be.reshape(2, P).T.copy(),
        "ones1": np.ones((1, P), np.float32),
        "onecol": np.ones((P, 1), np.float32),
        "tokid": j[:, None] + 128.0 * c4[None, :],
    }
    pk = np.zeros((P, PACKF_COLS), np.float32)
    c0 = 0
    for name, rows, cols in _PACKF:
        pk[:rows, c0 : c0 + cols] = valsf[name]
        c0 += cols
    consts["pconstf"] = pk
    valsh = {
        "slt4h": (c4[:, None] < c4[None, :]),
        "slt32h": (c32[:, None] < c32[None, :]),
        "identh": np.eye(P),
    }
    pkh = np.zeros((P, PACKH_COLS), np.float16)
    c0 = 0
    for name, rows, cols in _PACKH:
        pkh[:rows, c0 : c0 + cols] = valsh[name].astype(np.float16)
        c0 += cols
    consts["pconsth"] = pkh
    return consts


def _chunk_range(ft):
    """Token chunks that can feed frame tile ft (dur in [1,8])."""
    return range(ft // 8, min(NCH - 1, ft) + 1)


def build_nc():
    nc = bacc.Bacc("TRN2", target_bir_lowering=False, debug=False, enable_asserts=False)

    enc_dr = nc.dram_tensor("enc", [BPC, S, H], F16, kind="ExternalInput")
    pit_dr = nc.dram_tensor("pitch", [BPC, S], F32, kind="ExternalInput")
    ene_dr = nc.dram_tensor("energy", [BPC, S], F32, kind="ExternalInput")
    dur_dr = nc.dram_tensor("durt", [BPC, S], F32, kind="ExternalInput")
    tab_dr = {
        nm: nc.dram_tensor(nm, [NBINS, H], BF16, kind="ExternalInput")
        for nm in ("dpt_hi", "det_hi")
    }
    pkf_dr = nc.dram_tensor("pconstf", [P, PACKF_COLS], F32, kind="ExternalInput")
    pkh_dr = nc.dram_tensor("pconsth", [P, PACKH_COLS], F16, kind="ExternalInput")
    out_dr = [
        nc.dram_tensor(f"out{b}", [T, H], F32, kind="ExternalOutput")
        for b in range(BPC)
    ]
    delta_dr = [[nc.dram_tensor(f"delta{b}_{h}", [DELTA_N, 1], F32) for h in range(2)]
                for b in range(BPC)]
    idxr_dr = [nc.dram_tensor(f"idxr{b}", [1, T], F16) for b in range(BPC)]

    with tile.TileContext(nc) as tc:
        with (
            tc.tile_pool(name="const", bufs=1) as cp,
            tc.tile_pool(name="work", bufs=2) as wp,
            tc.tile_pool(name="inb", bufs=1) as ib,
            tc.tile_pool(name="ytil", bufs=3) as yp,
            tc.tile_pool(name="cmat", bufs=3) as cpl,
            tc.tile_pool(name="gat", bufs=3) as gp,
            tc.tile_pool(name="pout", bufs=2, space="PSUM") as po,
            tc.tile_pool(name="peps", bufs=2, space="PSUM") as pe,
            tc.tile_pool(name="psmall", bufs=2, space="PSUM") as psm,
        ):
            # ---- constants (one packed DMA per dtype) ----
            csb = {}
            pkf_sb = cp.tile([P, PACKF_COLS], F32, tag="pconstf")
            nc.sync.dma_start(out=pkf_sb[:], in_=pkf_dr[:])
            c0 = 0
            for name, rows, cols in _PACKF:
                csb[name] = pkf_sb[0:rows, c0 : c0 + cols]
                c0 += cols
            pkh_sb = cp.tile([P, PACKH_COLS], F16, tag="pconsth")
            nc.sync.dma_start(out=pkh_sb[:], in_=pkh_dr[:])
            c0 = 0
            for name, rows, cols in _PACKH:
                csb[name] = pkh_sb[0:rows, c0 : c0 + cols]
                c0 += cols
            for nm, dr in tab_dr.items():
                t_ = cp.tile([P, 2, H], BF16, tag=nm)
                nc.sync.dma_start(
                    out=t_[:], in_=dr[:].rearrange("(h p) f -> p h f", p=P)
                )
                csb[nm] = t_
            zt = cp.tile([24, 171], F32)  # 24*171 == DELTA_N
            nc.gpsimd.memset(zt[:], 0.0)
            for b in range(BPC):
                for h in range(2):
                    nc.gpsimd.dma_start(
                        out=delta_dr[b][h][:].rearrange("(r q) o -> r (q o)", r=24),
                        in_=zt[:],
                    )

            # ---- early input prefetch; zeros + dur first on sync queue ----
            enc_sb, vp_reps, ve_reps, dur_raws = {}, {}, {}, {}
            for b in range(BPC):
                dr_ = ib.tile([NCH, P], F32, tag=f"dw{b}")
                nc.sync.dma_start(
                    out=dr_[:], in_=dur_dr[b].rearrange("(c p) -> c p", p=P)
                )
                dur_raws[b] = dr_

            for b in range(BPC):
                e_ = ib.tile([P, NCH, H], F16, tag=f"enc{b}")
                nc.sync.dma_start(
                    out=e_[:], in_=enc_dr[b].rearrange("(c p) f -> p c f", p=P)
                )
                enc_sb[b] = e_
            for b in range(BPC):
                vp_ = ib.tile([P, S], F32, tag=f"vp{b}")
                ve_ = ib.tile([P, S], F32, tag=f"ve{b}")
                nc.sync.dma_start(
                    out=vp_[:], in_=pit_dr[b][None, :].to_broadcast([P, S])
                )
                nc.sync.dma_start(
                    out=ve_[:], in_=ene_dr[b][None, :].to_broadcast([P, S])
                )
                vp_reps[b], ve_reps[b] = vp_, ve_

            c_tiles = {}
            y_tiles = {}

            def phase0(b):
                # ---- dur = max(round_half_even(durt), 1), [4, 128] fp16 ----
                MAGIC = float(1 << 23)
                dr0 = wp.tile([NCH, P], F32, tag="dr0")
                nc.vector.tensor_scalar(out=dr0[:], in0=dur_raws[b][:], scalar1=MAGIC, scalar2=MAGIC, op0=A.add, op1=A.subtract)
                dur4 = wp.tile([NCH, P], F16, tag="dur")
                nc.vector.tensor_scalar(out=dur4[:], in0=dr0[:], scalar1=1.0, scalar2=None, op0=A.max)

                # ---- per-row inclusive scan; cross-row offsets via matmul ----
                loc = wp.tile([NCH, P], F16, tag="loc")
                nc.vector.tensor_tensor_scan(out=loc[:], data0=dur4[:], data1=dur4[:], initial=0.0, op0=A.add, op1=A.bypass)
                # off_row[0, c] = sum_{k<c} rowtot[k]  (rowtot = loc[:, 127])
                offr_ps = psm.tile([1, NCH], F32, tag="small")
                nc.tensor.matmul(out=offr_ps[:], lhsT=loc[:, P - 1 : P], rhs=csb["slt4h"], start=True, stop=True)
                off_sb = wp.tile([1, NCH], F32, tag="off4")
                nc.vector.tensor_copy(out=off_sb[:], in_=offr_ps[:])
                # cum[p, c] = loc[c, p] + off[c]
                cum_ps = psm.tile([P, NCH], F32, tag="small")
                nc.tensor.matmul(out=cum_ps[:], lhsT=loc[:], rhs=csb["identh"][0:NCH, 0:NCH], start=True, stop=False)
                nc.tensor.matmul(out=cum_ps[:], lhsT=csb["ones1"], rhs=off_sb[:], start=False, stop=True)
                cum_i32 = wp.tile([P, NCH], I32, tag="cumi")
                nc.vector.tensor_copy(out=cum_i32[:], in_=cum_ps[:])

                # ---- delta: scatter 1.0 at cum positions (zeroed up-front) ----
                for c in range(NCH):
                    nc.gpsimd.indirect_dma_start(
                        out=delta_dr[b][c // 2][:],
                        out_offset=IndirectOffsetOnAxis(ap=cum_i32[:, c : c + 1], axis=0),
                        in_=csb["onecol"],
                        in_offset=None,
                    )

            def phase_idx(b):
                # ---- frame_idx = inclusive prefix of delta, [32, 128] fp16;
                # the scan sums both half-delta streams as it scans ----
                d32a = wp.tile([32, P], F32, tag="d32a")
                nc.scalar.dma_start(
                    out=d32a[:],
                    in_=delta_dr[b][0][0:T, :].rearrange("(r p) o -> r (p o)", p=P),
                )
                d32b = wp.tile([32, P], F32, tag="d32b")
                nc.scalar.dma_start(
                    out=d32b[:],
                    in_=delta_dr[b][1][0:T, :].rearrange("(r p) o -> r (p o)", p=P),
                )
                loc = wp.tile([32, P], F16, tag="floc")
                nc.vector.tensor_tensor_scan(out=loc[:], data0=d32a[:], data1=d32b[:], initial=0.0, op0=A.add, op1=A.add)
                # off_col[r, 0] = sum_{k<r} rowtot[k]
                offc_ps = psm.tile([32, 1], F32, tag="small")
                nc.tensor.matmul(out=offc_ps[:], lhsT=csb["slt32h"], rhs=loc[:, P - 1 : P], start=True, stop=True)
                offc_sb = wp.tile([32, 1], F32, tag="off32")
                nc.vector.tensor_copy(out=offc_sb[:], in_=offc_ps[:])
                idxT = wp.tile([32, P], F16, tag="idxT")
                nc.vector.tensor_scalar(out=idxT[:], in0=loc[:], scalar1=offc_sb[:], scalar2=None, op0=A.add)
                # assemble [1, 4096] row in DRAM, then replicate to 128 partitions
                nc.scalar.dma_start(
                    out=idxr_dr[b][0:1, :].rearrange("o (r p) -> o r p", p=P),
                    in_=idxT[:],
                )
                bc_all = wp.tile([P, T], F16, tag="bca")
                nc.scalar.dma_start(
                    out=bc_all[:], in_=idxr_dr[b][0][None, :].to_broadcast([P, T])
                )
                # ---- one-hot C per token chunk ----
                for k in range(NCH):
                    ftlo, fthi = k, min(8 * k + 7, 31)
                    span = (fthi - ftlo + 1) * P
                    ct = cpl.tile([P, span], F16, tag=f"ck{k}")
                    nc.vector.tensor_scalar(
                        out=ct[:], in0=bc_all[:, ftlo * P : (fthi + 1) * P],
                        scalar1=csb["tokid"][:, k : k + 1], scalar2=None,
                        op0=A.is_equal,
                    )
                    c_tiles[(b, k)] = ct

            cmp_tiles = {}

            def phase_cmp(b):
                # ---- C matrices: C[bin_p, tok] = (boundary[bin] < v[tok]) ----
                cp0 = ib.tile([P, S], BF16, tag=f"cp0_{b}")
                cp1 = ib.tile([P, S], BF16, tag=f"cp1_{b}")
                nc.vector.tensor_scalar(out=cp0[:], in0=vp_reps[b][:], scalar1=csb["bndp"][:, 0:1], scalar2=None, op0=A.is_gt)
                nc.vector.tensor_scalar(out=cp1[:], in0=vp_reps[b][:], scalar1=csb["bndp"][:, 1:2], scalar2=None, op0=A.is_gt)
                ce0 = ib.tile([P, S], BF16, tag=f"ce0_{b}")
                ce1 = ib.tile([P, S], BF16, tag=f"ce1_{b}")
                nc.vector.tensor_scalar(out=ce0[:], in0=ve_reps[b][:], scalar1=csb["bnde"][:, 0:1], scalar2=None, op0=A.is_gt)
                nc.vector.tensor_scalar(out=ce1[:], in0=ve_reps[b][:], scalar1=csb["bnde"][:, 1:2], scalar2=None, op0=A.is_gt)
                cmp_tiles[b] = (cp0, cp1, ce0, ce1)

            def phase_y(b):
                # ---- y = (enc + base) + dtab telescoping, fp16 ----
                cp0, cp1, ce0, ce1 = cmp_tiles[b]
                y_sb = yp.tile([P, NCH, H], F16, tag="y")
                for c in range(NCH):
                    eps = pe.tile([P, H], F32, tag="eps")
                    nc.tensor.matmul(out=eps[:], lhsT=cp0[:, ts(c, P)], rhs=csb["dpt_hi"][:, 0, :], start=True, stop=False)
                    nc.tensor.matmul(out=eps[:], lhsT=cp1[:, ts(c, P)], rhs=csb["dpt_hi"][:, 1, :], start=False, stop=False)
                    nc.tensor.matmul(out=eps[:], lhsT=ce0[:, ts(c, P)], rhs=csb["det_hi"][:, 0, :], start=False, stop=False)
                    nc.tensor.matmul(out=eps[:], lhsT=ce1[:, ts(c, P)], rhs=csb["det_hi"][:, 1, :], start=False, stop=False)
                    nc.tensor.matmul(out=eps[:], lhsT=csb["identh"], rhs=enc_sb[b][:, c, :], start=False, stop=True)
                    if c % 2 == 0:
                        nc.vector.tensor_copy(out=y_sb[:, c, :], in_=eps[:])
                    else:
                        nc.scalar.activation(out=y_sb[:, c, :], in_=eps[:], func=AF.Copy)
                y_tiles[b] = y_sb

            def phase_out(b, g4s=(0, 1, 2, 3)):
                # ---- out[ft*128+p, :] = y[idx, :] via one-hot fp16 matmuls ----
                y_sb = y_tiles[b]
                for g4 in g4s:  # 1024-frame store groups
                    gbuf = gp.tile([P, 8, H], F32, tag="g")
                    for q in range(2):  # quads of frame tiles
                        out_ps = po.tile([P, 1024], F32, tag="out")
                        for half in range(4):
                            ft = g4 * 8 + q * 4 + half
                            ks = list(_chunk_range(ft))
                            for j, k in enumerate(ks):
                                nc.tensor.matmul(
                                    out=out_ps[:, half * H : (half + 1) * H],
                                    lhsT=c_tiles[(b, k)][:, (ft - k) * P : (ft - k + 1) * P],
                                    rhs=y_sb[:, k, :],
                                    start=(j == 0), stop=(j == len(ks) - 1),
                                )
                        if (g4 * 2 + q) % 3 == 0:
                            nc.vector.tensor_copy(out=gbuf[:, 4 * q : 4 * q + 4, :], in_=out_ps[:])
                        else:
                            nc.scalar.activation(
                                out=gbuf[:, 4 * q : 4 * q + 4, :], in_=out_ps[:], func=AF.Copy
                            )
                    if b == BPC - 1:
                        nc.sync.dma_start(
                            out=out_dr[b][g4 * 1024 : g4 * 1024 + 512, :].rearrange(
                                "(c p) f -> p c f", p=P
                            ),
                            in_=gbuf[:, 0:4, :],
                        )
                        nc.scalar.dma_start(
                            out=out_dr[b][g4 * 1024 + 512 : (g4 + 1) * 1024, :].rearrange(
                                "(c p) f -> p c f", p=P
                            ),
                            in_=gbuf[:, 4:8, :],
                        )
                    else:
                        nc.sync.dma_start(
                            out=out_dr[b][g4 * 1024 : (g4 + 1) * 1024, :].rearrange(
                                "(c p) f -> p c f", p=P
                            ),
                            in_=gbuf[:],
                        )

            for b in range(BPC):
                phase0(b)
            phase_cmp(0)
            phase_y(0)
            phase_idx(0)
            phase_cmp(1)
            phase_y(1)
            phase_idx(1)
            phase_out(0)
            phase_cmp(2)
            phase_y(2)
            phase_idx(2)
            phase_out(1)
            phase_cmp(3)
            phase_y(3)
            phase_idx(3)
            phase_out(2)
            phase_out(3)

    nc.compile()
    return nc


_NC_CACHE = {}


def _get_nc():
    if "nc" not in _NC_CACHE:
        _NC_CACHE["nc"] = build_nc()
    return _NC_CACHE["nc"]


def make_in_maps(inputs):
    enc = np.asarray(inputs["encoder_output"], np.float32)
    pit = np.ascontiguousarray(np.asarray(inputs["pitch_target"], np.float32))
    ene = np.ascontiguousarray(np.asarray(inputs["energy_target"], np.float32))
    dur = np.ascontiguousarray(np.asarray(inputs["duration_target"], np.float32))
    ptab = np.asarray(inputs["pitch_table"], np.float32)
    etab = np.asarray(inputs["energy_table"], np.float32)
    # fold the telescoping-sum base row into enc on the host
    enc = np.ascontiguousarray((enc + (ptab[0] + etab[0])[None, None, :]).astype(np.float16))
    consts = _host_constants(ptab, etab)
    in_maps = []
    for c in range(NCORES):
        sl = slice(c * BPC, (c + 1) * BPC)
        m = dict(consts)
        m["enc"] = enc[sl]
        m["pitch"] = pit[sl]
        m["energy"] = ene[sl]
        m["durt"] = dur[sl]
        in_maps.append(m)
    return in_maps


def run(inputs, trace=False):
    nc = _get_nc()
    in_maps = make_in_maps(inputs)
    res = run_bass_kernel_spmd(nc, in_maps, list(range(NCORES)), trace=trace)
    out = np.empty((B, T, H), np.float32)
    for c in range(NCORES):
        for b in range(BPC):
            out[c * BPC + b] = res.results[c][f"out{b}"]
    return out, res


def kernel(**inputs):
    out, _ = run(inputs, trace=False)
    return out
